# revision 13
# baseline (speedup 1.0000x reference)
"""Trainium2 Bass kernel for nn_CustomRNN: batched Elman RNN.

  h_t = tanh(x_t @ Wx + b_ih + h_{t-1} @ Wh);  out = h_S @ W_ho + b_ho

Strategy:
  * Data-parallel over batch: 512 rows -> 8 cores x 64 rows.
  * The recurrence is strongly contracting (spectral radius of Wh ~0.92,
    further damped by tanh'), so h_S depends only on the last ~40-128
    timesteps to below fp32 precision.  A cheap fp64 CPU probe on 8 batch
    rows measures the actual truncation error and picks the shortest safe
    window Teff.
  * On-device scan keeps the hidden state TRANSPOSED and packed as
    hT[p, kb*64+b] = h[b, kb*128+p] so each step is a handful of matmuls
    into one PSUM bank plus a single ACT tanh (PSUM -> SBUF).  b_ih is
    folded in via an all-ones row augmented into the transposed x.
  * Matmuls run in fp16 (1 cyc/row on the PE, vs 4 for fp32) with fp32
    PSUM accumulation.  Because the contraction erases earlier rounding,
    only the last steps need more precision: the last TAIL steps use
    hi/lo split-fp16 weights/x, and the last HSPLIT steps additionally
    split the hidden state, recovering ~3e-5 relative accuracy overall.
  * x is pre-transposed/pre-split on the host (layout prep only, no FLOPs).
"""

import numpy as np

B, S, I, H, CLS = 512, 1024, 64, 256, 10
NCORES = 8
BLOC = B // NCORES  # 64 batch rows per core
TAIL = 12  # split-fp16 weights/x steps at the end of the scan
HSPLIT = 6  # of those, steps that also split the hidden state
CH = 8  # timesteps per streamed x chunk

_TEFF_LADDER = (36, 40, 48, 64, 96, 128, 192, 256, 384, 512, 1024)
_PROBE_TOL = 2.5e-5  # keeps truncation below the kernel's own ~4e-5
# fp16-with-split-tail noise, with margin for probe-sample variance


def _probe_scan(x, Wx, Wh, b_ih, t0):
    h = np.zeros((x.shape[0], H), np.float64)
    for t in range(t0, x.shape[1]):
        h = np.tanh(x[:, t] @ Wx + b_ih + h @ Wh)
    return h


def _pick_teff(x, Wx, Wh, b_ih):
    """Pick the shortest truncation window whose error is below fp32 noise.

    Compares truncated scans (h=0 start) on 8 batch rows at successive
    window lengths, in fp64 so probe rounding doesn't mask the result; the
    recurrence's contraction makes the gap between consecutive windows a
    sound bound on the truncation error.
    """
    xp = np.ascontiguousarray(x[:8], np.float64)
    Wx, Wh, b_ih = (np.asarray(a, np.float64) for a in (Wx, Wh, b_ih))
    cache = {}

    def h_for(teff):
        if teff not in cache:
            cache[teff] = _probe_scan(xp, Wx, Wh, b_ih, S - teff)
        return cache[teff]

    for i, teff in enumerate(_TEFF_LADDER[:-1]):
        a, b = h_for(teff), h_for(_TEFF_LADDER[i + 1])
        rel = np.abs(a - b).max() / (np.abs(b).max() + 1e-30)
        if rel < _PROBE_TOL:
            return teff
    return S


def _emit(tc, ctx, aps, teff, tail=TAIL, hsplit=HSPLIT):
    """Emit the per-core program.

    aps: dict of DRAM APs: xts/xtsl (fp16 transposed+augmented x, full +
    lo-residual tail), wxa/wxal, whr/whrl (fp16 weights hi/lo),
    whor (fp32), out.
    """
    import concourse.mybir as mybir

    nc = tc.nc
    f32 = mybir.dt.float32
    f16 = mybir.dt.float16
    Tanh = mybir.ActivationFunctionType.Tanh

    const = ctx.enter_context(tc.tile_pool(name="const", bufs=1))
    xch = ctx.enter_context(tc.tile_pool(name="xch", bufs=4))
    hpool = ctx.enter_context(tc.tile_pool(name="h", bufs=4))
    psum = ctx.enter_context(tc.tile_pool(name="psum", bufs=6, space="PSUM"))
    opsum = ctx.enter_context(tc.tile_pool(name="opsum", bufs=1, space="PSUM"))
    osb = ctx.enter_context(tc.tile_pool(name="osb", bufs=1))

    nch = max(1, teff // CH)
    chln = teff // nch
    xtiles = []

    def ensure_chunk(tt):
        c = tt // chln
        while len(xtiles) <= c:
            cc = len(xtiles)
            xt = xch.tile([128, chln * 64], f16, tag="xchunk")
            nc.sync.dma_start(
                xt[:], aps["xts"][:, cc * chln * 64 : (cc + 1) * chln * 64]
            )
            xtiles.append(xt)
        return xtiles[c]

    # PE warm-up: the HAM clock gate holds the PE at 1.2 GHz until it has
    # been busy for a ~3.4us window.  Dummy matmuls on a zeroed tile span
    # the initial DMA wait so the real scan starts at 2.4 GHz.
    warm = const.tile([128, 64], f16)
    nc.any.memset(warm[:], 0.0)
    wps = opsum.tile([64, 64], f32)  # reuses the output-psum pool's bank budget
    NWARM = 120
    for i in range(NWARM):
        nc.tensor.matmul(
            wps[:, :], warm[:, :], warm[:, :], start=(i == 0), stop=(i == NWARM - 1)
        )

    ensure_chunk(0)  # first compute chunk in flight before the weights
    wx = const.tile([128, 256], f16)
    nc.sync.dma_start(wx[:], aps["wxa"])
    wh = const.tile([128, 2, 256], f16)
    nc.sync.dma_start(wh[:], aps["whr"])
    wxl = const.tile([128, 256], f16)
    nc.sync.dma_start(wxl[:], aps["wxal"])
    whl = const.tile([128, 2, 256], f16)
    nc.sync.dma_start(whl[:], aps["whrl"])
    who = const.tile([128, 2, CLS], f32)
    nc.sync.dma_start(who[:], aps["whor"])
    xl_sb = const.tile([128, tail * 64], f16)
    nc.sync.dma_start(xl_sb[:], aps["xtsl"])

    LOOKAHEAD = 2  # x-projection matmuls run ahead to fill PE stalls
    psums = {}
    mm_state = {}

    def mm(t, out_sl, lhsT, rhs):
        n_mm = mm_state[t][1]
        k = mm_state[t][0]
        nc.tensor.matmul(out_sl, lhsT, rhs, start=(k == 0), stop=(k == n_mm - 1))
        mm_state[t][0] += 1

    def emit_xmms(tt):
        """PSUM tile + x-projection matmuls for step tt (h-independent)."""
        if tt >= teff or tt in psums:
            return
        xh = ensure_chunk(tt)[:, (tt % chln) * 64 : (tt % chln) * 64 + 64]
        x_split = tt >= teff - tail
        h_split = tt >= teff - hsplit
        wh_prods = 0 if tt == 0 else (3 if h_split else (2 if x_split else 1))
        # One accumulation group per PSUM bank: start on the first matmul,
        # stop on the last; the bank-wide pending-zero makes each region's
        # first writer overwrite and later ones accumulate.
        ps = psum.tile([128, 128], f32)
        psums[tt] = ps
        mm_state[tt] = [0, (6 if x_split else 2) + 4 * wh_prods]
        for jb in range(2):
            osl = ps[:, jb * 64 : jb * 64 + 64]
            jsl = slice(jb * 128, jb * 128 + 128)
            mm(tt, osl, wx[:, jsl], xh)
            if x_split:
                off = (tt - (teff - tail)) * 64
                mm(tt, osl, wx[:, jsl], xl_sb[:, off : off + 64])
                mm(tt, osl, wxl[:, jsl], xh)

    hTh = hTl = hf = None
    for t in range(teff):
        for tt in range(t, min(t + LOOKAHEAD + 1, teff)):
            emit_xmms(tt)
        ps = psums.pop(t)
        h_split = t >= teff - hsplit
        x_split = t >= teff - tail
        if t > 0:
            # hl-dependent products go last: h_lo comes off a DVE subtract,
            # so its latency hides behind the hh-product issue stream.
            for jb in range(2):
                osl = ps[:, jb * 64 : jb * 64 + 64]
                jsl = slice(jb * 128, jb * 128 + 128)
                for kb in range(2):
                    ksl = slice(kb * 64, kb * 64 + 64)
                    mm(t, osl, wh[:, kb, jsl], hTh[:, ksl])
                    if x_split:
                        mm(t, osl, whl[:, kb, jsl], hTh[:, ksl])
            if h_split:
                for jb in range(2):
                    osl = ps[:, jb * 64 : jb * 64 + 64]
                    jsl = slice(jb * 128, jb * 128 + 128)
                    for kb in range(2):
                        ksl = slice(kb * 64, kb * 64 + 64)
                        mm(t, osl, wh[:, kb, jsl], hTl[:, ksl])
        assert mm_state[t][0] == mm_state[t][1], (t, mm_state[t])

        # producer: the step before an h-split step (and the final step)
        # must expose fp32 h; h-split consumers also need hi/lo fp16 parts.
        if t == teff - 1:
            hf = hpool.tile([128, 128], f32, tag="hf")
            nc.scalar.activation(hf[:], ps[:], Tanh)
        elif t + 1 >= teff - hsplit:
            hf = hpool.tile([128, 128], f32, tag="hf")
            nc.scalar.activation(hf[:], ps[:], Tanh)
            hTh = hpool.tile([128, 128], f16, tag="hh")
            nc.vector.tensor_copy(hTh[:], hf[:])
            hTl = hpool.tile([128, 128], f16, tag="hl")
            nc.vector.tensor_tensor(hTl[:], hf[:], hTh[:], mybir.AluOpType.subtract)
        else:
            hTh = hpool.tile([128, 128], f16, tag="hh")
            nc.scalar.activation(hTh[:], ps[:], Tanh)

    ops = opsum.tile([64, CLS], f32)
    nc.tensor.matmul(ops[:, :], hf[:, 0:64], who[:, 0, :], start=True, stop=False)
    nc.tensor.matmul(ops[:, :], hf[:, 64:128], who[:, 1, :], start=False, stop=True)
    ob = osb.tile([64, CLS], f32)
    nc.vector.tensor_copy(ob[:], ops[:])
    nc.sync.dma_start(aps["out"], ob[:])


def _build(teff, tail=TAIL, hsplit=HSPLIT):
    from contextlib import ExitStack

    import concourse.mybir as mybir
    import concourse.tile as tile
    from concourse import bacc

    f32 = mybir.dt.float32
    f16 = mybir.dt.float16
    nc = bacc.Bacc("TRN2", target_bir_lowering=False, debug=False)
    t = {}
    t["xts"] = nc.dram_tensor("xts", [128, teff * 64], f16, kind="ExternalInput")
    t["xtsl"] = nc.dram_tensor("xtsl", [128, tail * 64], f16, kind="ExternalInput")
    t["wxa"] = nc.dram_tensor("wxa", [128, 256], f16, kind="ExternalInput")
    t["wxal"] = nc.dram_tensor("wxal", [128, 256], f16, kind="ExternalInput")
    t["whr"] = nc.dram_tensor("whr", [128, 2, 256], f16, kind="ExternalInput")
    t["whrl"] = nc.dram_tensor("whrl", [128, 2, 256], f16, kind="ExternalInput")
    t["whor"] = nc.dram_tensor("whor", [128, 2, CLS], f32, kind="ExternalInput")
    t["out"] = nc.dram_tensor("out", [BLOC, CLS], f32, kind="ExternalOutput")

    with tile.TileContext(nc) as tc, ExitStack() as ctx:
        _emit(tc, ctx, {k: v.ap() for k, v in t.items()}, teff, tail, hsplit)
    nc.compile()
    return nc


_prog_cache = {}


def _split16(a):
    hi = a.astype(np.float16)
    lo = (a - hi.astype(np.float32)).astype(np.float16)
    return hi, lo


def _host_prep(inputs, teff, tail=TAIL):
    """Shard + lay out inputs for the device program (no FLOPs, layout only)."""
    x = np.asarray(inputs["inputs"], np.float32)
    W_ih = np.asarray(inputs["W_ih"], np.float32)
    b_ih = np.asarray(inputs["b_ih"], np.float32)
    W_ho = np.asarray(inputs["W_ho"], np.float32)

    wxa = np.zeros((128, H), np.float32)
    wxa[:I] = W_ih[:I]
    wxa[I] = b_ih  # bias enters via the all-ones row of xts
    wxa_h, wxa_l = _split16(wxa)
    whr = np.ascontiguousarray(
        W_ih[I:].reshape(2, 128, H).transpose(1, 0, 2)
    ).astype(np.float32)
    whr_h, whr_l = _split16(whr)
    whor = np.ascontiguousarray(W_ho.reshape(2, 128, CLS).transpose(1, 0, 2))

    in_maps = []
    for c in range(NCORES):
        xs = x[c * BLOC : (c + 1) * BLOC, S - teff :, :]  # [64, teff, 64]
        xts = np.zeros((128, teff * 64), np.float32)
        xts[:I] = xs.transpose(2, 1, 0).reshape(I, teff * BLOC)
        xts[I] = 1.0
        xts_h, xts_l = _split16(xts)
        in_maps.append(
            {
                "xts": xts_h,
                "xtsl": np.ascontiguousarray(xts_l[:, (teff - tail) * 64 :]),
                "wxa": wxa_h,
                "wxal": wxa_l,
                "whr": whr_h,
                "whrl": whr_l,
                "whor": whor,
            }
        )
    return in_maps


def kernel(**inputs):
    from concourse.bass_utils import run_bass_kernel_spmd

    W_ih = np.asarray(inputs["W_ih"], np.float32)
    b_ih = np.asarray(inputs["b_ih"], np.float32)
    b_ho = np.asarray(inputs["b_ho"], np.float32)
    x = np.asarray(inputs["inputs"], np.float32)

    teff = _pick_teff(x, W_ih[:I], W_ih[I:], b_ih)
    if teff not in _prog_cache:
        _prog_cache[teff] = _build(teff)
    nc = _prog_cache[teff]

    in_maps = _host_prep(inputs, teff)
    res = run_bass_kernel_spmd(nc, in_maps, list(range(NCORES)))
    out = np.concatenate([res.results[c]["out"] for c in range(NCORES)], axis=0)
    return (out + b_ho).astype(np.float32)
